# revision 15
# baseline (speedup 1.0000x reference)
"""Causal self-attention on 8 NeuronCores (TRN2), batch x head-group hybrid.

Reference: y = proj(softmax(causal(Q K^T / sqrt(64))) V) with
B=4, T=2048, D=1024, H=16 heads, head_dim=64.

Sharding: core c owns batch c//2 and head-group c%2 (8 heads = 4 head
pairs). Each core reads only its batch's x (pre-packed bf16 on host),
computes QKV for its 512 qkv columns, runs attention for its 4 head
pairs, and projects through its 512 rows of w_proj, emitting a
[T, D] bf16 partial. The host sums the 2 partials per batch.

All matmuls run in bf16 with fp32 PSUM accumulation. The attention
inner loop is paced by the scalar-engine exp (~1.1us per 128x1024
tile vs ~0.8us of PE work per kk step), so the pairs are software-
pipelined: pair p+1's QKV projection m-groups (and, for the last
pair, the output-projection tiles) are interleaved into pair p's
attention stream to keep the PE busy during exp latency.
"""

import sys

for _p in ("/opt/trn_rl_repo",):
    if _p not in sys.path:
        sys.path.insert(0, _p)

import ml_dtypes
import numpy as np

import concourse.bass as bass
import concourse.bacc as bacc
import concourse.mybir as mybir
from concourse import tile
from concourse.bass_utils import run_bass_kernel_spmd
from concourse.masks import make_identity

B, T, D, H = 4, 2048, 1024, 16
HD = D // H           # 64 head dim
NCORES = 8
GH = 8                # heads per core (head group)
NP = GH // 2          # 4 head pairs per core
CW = GH * HD          # 512: per-core qkv column slice width per matrix
KC = D // 128         # 8 contraction chunks for the qkv projection
NM = 3 * CW // 128    # 12 qkv projection m-tiles (4 Q, 4 K, 4 V pairs)
NCH = T // 512        # 4 token chunks
NQ = 512              # query chunk
NG = NQ // 128        # 4 key-tiles per S^T group
NJQ = T // NQ         # 4 query chunks
NSTEP = NG * NJQ * (NJQ + 1) // 2   # 40 kk steps per pair
F32 = mybir.dt.float32
F32R = mybir.dt.float32r
BF16 = mybir.dt.bfloat16
EXP = mybir.ActivationFunctionType.Exp

VST = 2 * (HD + 1)    # 130: per-pair V tile stride (per head: 64 cols + ones)


def build_kernel():
    nc = bacc.Bacc("TRN2", target_bir_lowering=False, debug=False)

    # host-packed layouts (see make_in_maps):
    #   xb [128, ch, kc, 512]  bf16 -- x[b]^T chunked for the qkv matmuls
    #   wq [128, m, kc, 128]   bf16 -- qkv weight m-tiles (m: 4 Q, 4 K, 4 V)
    #   wp [128, pair, 1024]   bf16 -- w_proj rows for this head group
    xb = nc.dram_tensor("xb", [128, NCH * KC * NQ], BF16, kind="ExternalInput")
    wq = nc.dram_tensor("wq", [128, NM * KC * 128], BF16, kind="ExternalInput")
    wp = nc.dram_tensor("wp", [128, NP * D], BF16, kind="ExternalInput")
    out = nc.dram_tensor("out", [T, D], BF16, kind="ExternalOutput")

    with tile.TileContext(nc) as tc:
        _body(tc, xb.ap(), wq.ap(), wp.ap(), out.ap())
    nc.compile()
    return nc


def _body(tc, xb, wq, wp, out):
    nc = tc.nc
    with (
        tc.tile_pool(name="const", bufs=1) as const,
        tc.tile_pool(name="vs", bufs=2) as vspool,
        tc.tile_pool(name="pt", bufs=3) as ptpool,
        tc.tile_pool(name="ytu", bufs=2) as ytupool,
        tc.tile_pool(name="dn", bufs=2) as dnpool,
        tc.tile_pool(name="os", bufs=2) as ospool,
        tc.tile_pool(name="pst", bufs=2, space="PSUM") as pst,
        tc.tile_pool(name="pav", bufs=1, space="PSUM") as pav,
        tc.tile_pool(name="psm", bufs=2, space="PSUM") as psm,
    ):
        # ---- persistent tiles (per-pair tensors are separate tiles so the
        # dependency tracker never serializes pair p's attention against
        # pair p+1's projection writes) ----
        wq_sb = const.tile([128, NM, KC, 128], BF16, tag="wq")
        xb_sb = const.tile([128, NCH, KC, NQ], BF16, tag="xb")
        wp_sb = const.tile([128, NP, D], BF16, tag="wp")
        qt = [const.tile([128, T], BF16, tag=f"qt{p}", name=f"qt{p}")
              for p in range(NP)]
        kt = [const.tile([128, T], BF16, tag=f"kt{p}", name=f"kt{p}")
              for p in range(NP)]
        vb = [const.tile([128, T // 128, VST], BF16, tag=f"vb{p}",
                         name=f"vb{p}") for p in range(NP)]
        yt = [const.tile([128, T], BF16, tag=f"yt{p}", name=f"yt{p}")
              for p in range(NP)]
        ident = const.tile([128, 128], BF16, tag="ident")
        ones = const.tile([128, 128], BF16, tag="ones")

        wqv = wq.rearrange("p (m k t) -> p m k t", m=NM, k=KC)
        xbv = xb.rearrange("p (c k t) -> p c k t", c=NCH, k=KC)
        # the first qkv m-group needs wq m0 + xb ch0; split ch0 by kc halves
        # so the PE can start after ~1.5MB of transfer
        nc.sync.dma_start(wq_sb[:, 0], wqv[:, 0])
        nc.sync.dma_start(xb_sb[:, 0, 0:4], xbv[:, 0, 0:4])
        nc.sync.dma_start(xb_sb[:, 0, 4:8], xbv[:, 0, 4:8])
        for m in range(1, NM):
            nc.sync.dma_start(wq_sb[:, m], wqv[:, m])
        for c in range(1, NCH):
            nc.sync.dma_start(xb_sb[:, c], xbv[:, c])
        nc.sync.dma_start(wp_sb[:], wp.rearrange("p (h t) -> p h t", h=NP))

        make_identity(nc, ident[:])
        nc.gpsimd.memset(ones[:], 1.0)
        # ones columns (denominator trick): col 65*j + 64 of every vb tile
        for p in range(NP):
            onesv = bass.AP(
                vb[p].tensor, vb[p][:].offset + HD,
                [vb[p][:].ap[0], [HD + 1, 32]],
            )
            nc.vector.tensor_copy(onesv, ones[:, 0:32])
        scale = 1.0 / float(np.sqrt(HD))

        def qkv_group(p, which, ch):
            # one m-group: 8-deep contraction into one PSUM bank + evacuation
            m = which * NP + p
            t0 = ch * NQ
            ps = psm.tile([128, NQ], F32, tag="ps")
            for kc in range(KC):
                nc.tensor.matmul(
                    ps[:],
                    wq_sb[:, m, kc, :],
                    xb_sb[:, ch, kc, :],
                    start=(kc == 0),
                    stop=(kc == KC - 1),
                )
            if which == 0:
                nc.vector.tensor_copy(qt[p][:, t0 : t0 + NQ], ps[:])
            elif which == 1:
                nc.vector.tensor_copy(kt[p][:, t0 : t0 + NQ], ps[:])
            else:
                vs = vspool.tile([128, NQ], BF16, tag="vs")
                nc.vector.tensor_copy(vs[:], ps[:])
                pt2 = psm.tile([128, NQ], BF16, tag="ps")
                for q in range(NG):
                    nc.tensor.transpose(
                        pt2[:, q * 128 : (q + 1) * 128],
                        vs[:, q * 128 : (q + 1) * 128],
                        ident[:],
                    )
                # pt2: [tok 128][tile q: h0 64 | h1 64] -> vb slots
                dstv = bass.AP(
                    vb[p].tensor,
                    vb[p][:].offset + ch * NG * VST,
                    [vb[p][:].ap[0], [VST, NG], [HD + 1, 2], [1, HD]],
                )
                srcv = pt2[:].rearrange("p (t h d) -> p t h d", t=NG, h=2)
                nc.vector.tensor_copy(dstv, srcv)

        def qkv_feed(p):
            # thunks for pair p's projection, Q/K first (attention consumes
            # them first), usable from jq 0
            return [
                (0, lambda w=w, c=c: qkv_group(p, w, c))
                for c in range(NCH)
                for w in (0, 1, 2)
            ]

        def finalize_norm(p, jq, ytu, avs=None):
            # divide O^T rows by the denominator row (broadcast to 64 parts);
            # the custom-DVE reciprocal and partition_broadcast only operate
            # on physical partition 0, so first move the denominator row
            # (partition 64) down with a plain scalar-engine copy
            q0 = jq * NQ
            dn0 = dnpool.tile([1, 2 * NQ], F32, tag="dn0")
            if avs is None:
                nc.scalar.copy(dn0[:], ytu[HD : HD + 1, :])
            else:
                # tail path: read the denominators straight from PSUM so the
                # scalar copy overlaps the vector ytu evacuation
                for h in range(2):
                    nc.scalar.copy(dn0[:, h * NQ : (h + 1) * NQ],
                                   avs[h][HD : HD + 1, :])
            dnr0 = dnpool.tile([1, 2 * NQ], F32, tag="dnr0")
            nc.vector.reciprocal_approx_fast(dnr0[:], dn0[:])
            dnr = dnpool.tile([HD, 2 * NQ], F32, tag="dnr")
            nc.gpsimd.partition_broadcast(dnr[:], dnr0[:])
            for h in range(2):
                nc.vector.tensor_mul(
                    yt[p][h * HD : (h + 1) * HD, q0 : q0 + NQ],
                    ytu[0:HD, h * NQ : (h + 1) * NQ],
                    dnr[:, h * NQ : (h + 1) * NQ],
                )

        def out_tile(tt):
            # one 128-token slice of the output projection
            os_ = ospool.tile([128, D], BF16, tag="os")
            for nn in range(D // NQ):
                pp = psm.tile([128, NQ], F32, tag="ps")
                for p in range(NP):
                    nc.tensor.matmul(
                        pp[:],
                        yt[p][:, tt * 128 : (tt + 1) * 128],
                        wp_sb[:, p, nn * NQ : (nn + 1) * NQ],
                        start=(p == 0),
                        stop=(p == NP - 1),
                    )
                nc.vector.tensor_copy(os_[:, nn * NQ : (nn + 1) * NQ], pp[:])
            nc.sync.dma_start(out[tt * 128 : (tt + 1) * 128, :], os_[:])

        def attention(p, feed, last=False):
            # Both heads of the pair processed together per kk-tile: h0 in
            # SBUF partitions 0-63, h1 in 64-127, so the S^T matmul pairs
            # land on PE row-tiles (64,128)@(0,0) and @(64,0) and overlap.
            # `feed` is a list of (min_jq, thunk): independent PE work
            # interleaved between kk steps to cover the exp latency.
            fi = 0
            gstep = 0
            nfeed = len(feed)
            for jq in range(NJQ):
                q0 = jq * NQ
                av0 = pav.tile([128, NQ], F32, tag="av0")
                av1 = pav.tile([128, NQ], F32, tag="av1")
                avs = [av0, av1]
                nkk = NG * (jq + 1)
                diag0 = NG * jq

                def s_exp(kk):
                    # S^T for both heads, then exp and the diagonal mask
                    i = kk - diag0          # >= 0 on the diagonal run
                    c0 = max(i, 0) * 128    # first valid q col in this chunk
                    w = NQ - c0
                    st = pst.tile([128, 2 * NQ], F32, tag="st")
                    for h in range(2):
                        nc.tensor.matmul(
                            st[:, h * NQ + c0 : (h + 1) * NQ],
                            kt[p][h * HD : (h + 1) * HD,
                                  kk * 128 : (kk + 1) * 128],
                            qt[p][h * HD : (h + 1) * HD, q0 + c0 : q0 + NQ],
                            start=True,
                            stop=True,
                        )
                    ptk = ptpool.tile([128, 2 * NQ], BF16, tag="pt")
                    stv = bass.AP(st.tensor, st[:].offset + c0,
                                  [st[:].ap[0], [NQ, 2], [1, w]])
                    ptv = bass.AP(ptk.tensor, ptk[:].offset + c0,
                                  [ptk[:].ap[0], [NQ, 2], [1, w]])
                    nc.scalar.activation(ptv, stv, EXP, scale=scale)
                    if i >= 0:
                        # zero q < kpart inside the 128-wide diagonal block
                        tri = bass.AP(ptk.tensor, ptk[:].offset + c0,
                                      [ptk[:].ap[0], [NQ, 2], [1, 128]])
                        nc.gpsimd.affine_select(
                            out=tri,
                            in_=tri,
                            pattern=[[0, 2], [1, 128]],
                            channel_multiplier=-1,
                            base=0,
                            compare_op=mybir.AluOpType.is_ge,
                            fill=0.0,
                        )
                    return ptk, c0

                # software pipeline: S(kk+1) is emitted before AV(kk) so the
                # PE has independent work while exp(kk) is in flight
                pend = s_exp(0)
                for kk in range(nkk):
                    nxt = s_exp(kk + 1) if kk + 1 < nkk else None
                    ptk, c0 = pend
                    for h in range(2):
                        nc.tensor.matmul(
                            avs[h][0 : HD + 1, c0:NQ],
                            vb[p][:, kk, h * (HD + 1) : (h + 1) * (HD + 1)],
                            ptk[:, h * NQ + c0 : (h + 1) * NQ],
                            start=(kk == 0),
                            stop=(kk == nkk - 1),
                        )
                    pend = nxt
                    gstep += 1
                    # pace the feed evenly over the 40 kk steps, honoring
                    # each thunk's earliest-jq window
                    while (fi < nfeed and feed[fi][0] <= jq
                           and gstep * nfeed + nfeed // 2 >= fi * NSTEP):
                        feed[fi][1]()
                        fi += 1
                # evacuate PSUM promptly (frees the av slots); rows 0..63 are
                # the unnormalized O^T, row 64 the denominator
                ytu = ytupool.tile([HD + 1, 2 * NQ], BF16, tag="ytu")
                for h in range(2):
                    nc.vector.tensor_copy(
                        ytu[:, h * NQ : (h + 1) * NQ], avs[h][0 : HD + 1, :]
                    )
                finalize_norm(p, jq, ytu,
                              avs=avs if (last and jq == NJQ - 1) else None)
                # flush feed thunks whose window just opened
                while (fi < nfeed and feed[fi][0] <= jq + 1
                       and gstep * nfeed + nfeed // 2 >= fi * NSTEP):
                    feed[fi][1]()
                    fi += 1
            while fi < nfeed:
                feed[fi][1]()
                fi += 1

        # ---- schedule ----
        # prologue: pair 0's projection
        for (_, thunk) in qkv_feed(0):
            thunk()
        # pair p's attention carries pair p+1's projection; the last pair
        # carries the output projection (tt 4j..4j+3 opens once pair 3's
        # query chunk j is normalized)
        for p in range(NP - 1):
            attention(p, qkv_feed(p + 1))
        attention(
            NP - 1,
            [(tt // 4 + 1, lambda tt=tt: out_tile(tt)) for tt in range(T // 128)],
            last=True,
        )


def make_in_maps(x, w_attn, w_proj):
    """Pack full fp32 inputs into per-core bf16 input maps."""
    bf = ml_dtypes.bfloat16
    x = np.asarray(x, dtype=np.float32)
    w_attn = np.asarray(w_attn, dtype=np.float32)
    w_proj = np.asarray(w_proj, dtype=np.float32)

    xbs = []
    for b in range(B):
        xT = x[b].T  # [D, T]
        xbs.append(
            np.ascontiguousarray(
                xT.reshape(KC, 128, NCH, NQ).transpose(1, 2, 0, 3)
            ).reshape(128, NCH * KC * NQ).astype(bf)
        )

    in_maps = []
    for c in range(NCORES):
        b, g = c // 2, c % 2
        c0 = g * CW
        wsl = np.concatenate(
            [w_attn[:, c0 : c0 + CW],
             w_attn[:, D + c0 : D + c0 + CW],
             w_attn[:, 2 * D + c0 : 2 * D + c0 + CW]],
            axis=1,
        )  # [D, 3*CW]
        wq = np.ascontiguousarray(
            wsl.reshape(KC, 128, NM, 128).transpose(1, 2, 0, 3)
        ).reshape(128, NM * KC * 128).astype(bf)
        wpc = np.ascontiguousarray(
            w_proj[c0 : c0 + CW, :].reshape(NP, 128, D).transpose(1, 0, 2)
        ).reshape(128, NP * D).astype(bf)
        in_maps.append({"xb": xbs[b], "wq": wq, "wp": wpc})
    return in_maps


_NC_CACHE = None


def kernel(x: np.ndarray, w_attn: np.ndarray, w_proj: np.ndarray) -> np.ndarray:
    global _NC_CACHE
    if _NC_CACHE is None:
        _NC_CACHE = build_kernel()
    nc = _NC_CACHE

    in_maps = make_in_maps(x, w_attn, w_proj)
    res = run_bass_kernel_spmd(nc, in_maps, core_ids=list(range(NCORES)))
    y = np.empty((B, T, D), dtype=np.float32)
    for b in range(B):
        y[b] = np.asarray(res.results[2 * b]["out"], np.float32) + np.asarray(
            res.results[2 * b + 1]["out"], np.float32
        )
    return y


if __name__ == "__main__":
    inputs = {
        "x": np.random.randn(B, T, D).astype(np.float32),
        "w_attn": (np.random.randn(D, 3 * D) / np.sqrt(D)).astype(np.float32),
        "w_proj": (np.random.randn(D, D) / np.sqrt(D)).astype(np.float32),
    }
    y = kernel(**inputs)
    print(y.shape, y.dtype)


# revision 16
# speedup vs baseline: 1.0082x; 1.0082x over previous
"""Causal self-attention on 8 NeuronCores (TRN2), batch x head-group hybrid.

Reference: y = proj(softmax(causal(Q K^T / sqrt(64))) V) with
B=4, T=2048, D=1024, H=16 heads, head_dim=64.

Sharding: core c owns batch c//2 and head-group c%2 (8 heads = 4 head
pairs). Each core reads only its batch's x (pre-packed bf16 on host),
computes QKV for its 512 qkv columns, runs attention for its 4 head
pairs, and projects through its 512 rows of w_proj, emitting a
[T, D] bf16 partial. The host sums the 2 partials per batch.

All matmuls run in bf16 with fp32 PSUM accumulation. The attention
inner loop is paced by the scalar-engine exp (~1.1us per 128x1024
tile vs ~0.8us of PE work per kk step), so the pairs are software-
pipelined: pair p+1's QKV projection m-groups (and, for the last
pair, the output-projection tiles) are interleaved into pair p's
attention stream to keep the PE busy during exp latency.
"""

import sys

for _p in ("/opt/trn_rl_repo",):
    if _p not in sys.path:
        sys.path.insert(0, _p)

import ml_dtypes
import numpy as np

import concourse.bass as bass
import concourse.bacc as bacc
import concourse.mybir as mybir
from concourse import tile
from concourse.bass_utils import run_bass_kernel_spmd
from concourse.masks import make_identity

B, T, D, H = 4, 2048, 1024, 16
HD = D // H           # 64 head dim
NCORES = 8
GH = 8                # heads per core (head group)
NP = GH // 2          # 4 head pairs per core
CW = GH * HD          # 512: per-core qkv column slice width per matrix
KC = D // 128         # 8 contraction chunks for the qkv projection
NM = 3 * CW // 128    # 12 qkv projection m-tiles (4 Q, 4 K, 4 V pairs)
NCH = T // 512        # 4 token chunks
NQ = 512              # query chunk
NG = NQ // 128        # 4 key-tiles per S^T group
NJQ = T // NQ         # 4 query chunks
NSTEP = NG * NJQ * (NJQ + 1) // 2   # 40 kk steps per pair
F32 = mybir.dt.float32
F32R = mybir.dt.float32r
BF16 = mybir.dt.bfloat16
EXP = mybir.ActivationFunctionType.Exp

VST = 2 * (HD + 1)    # 130: per-pair V tile stride (per head: 64 cols + ones)


def build_kernel():
    nc = bacc.Bacc("TRN2", target_bir_lowering=False, debug=False)

    # host-packed layouts (see make_in_maps):
    #   xb [128, ch, kc, 512]  bf16 -- x[b]^T chunked for the qkv matmuls
    #   wq [128, m, kc, 128]   bf16 -- qkv weight m-tiles (m: 4 Q, 4 K, 4 V)
    #   wp [128, pair, 1024]   bf16 -- w_proj rows for this head group
    xb = nc.dram_tensor("xb", [128, NCH * KC * NQ], BF16, kind="ExternalInput")
    wq = nc.dram_tensor("wq", [128, NM * KC * 128], BF16, kind="ExternalInput")
    wp = nc.dram_tensor("wp", [128, NP * D], BF16, kind="ExternalInput")
    out = nc.dram_tensor("out", [T, D], BF16, kind="ExternalOutput")

    with tile.TileContext(nc) as tc:
        _body(tc, xb.ap(), wq.ap(), wp.ap(), out.ap())
    nc.compile()
    return nc


def _body(tc, xb, wq, wp, out):
    nc = tc.nc
    with (
        tc.tile_pool(name="const", bufs=1) as const,
        tc.tile_pool(name="vs", bufs=2) as vspool,
        tc.tile_pool(name="pt", bufs=4) as ptpool,
        tc.tile_pool(name="ytu", bufs=2) as ytupool,
        tc.tile_pool(name="dn", bufs=2) as dnpool,
        tc.tile_pool(name="os", bufs=2) as ospool,
        tc.tile_pool(name="pst", bufs=2, space="PSUM") as pst,
        tc.tile_pool(name="pav", bufs=1, space="PSUM") as pav,
        tc.tile_pool(name="psm", bufs=2, space="PSUM") as psm,
    ):
        # ---- persistent tiles (per-pair tensors are separate tiles so the
        # dependency tracker never serializes pair p's attention against
        # pair p+1's projection writes) ----
        wq_sb = const.tile([128, NM, KC, 128], BF16, tag="wq")
        xb_sb = const.tile([128, NCH, KC, NQ], BF16, tag="xb")
        wp_sb = const.tile([128, NP, D], BF16, tag="wp")
        qt = [const.tile([128, T], BF16, tag=f"qt{p}", name=f"qt{p}")
              for p in range(NP)]
        kt = [const.tile([128, T], BF16, tag=f"kt{p}", name=f"kt{p}")
              for p in range(NP)]
        vb = [const.tile([128, T // 128, VST], BF16, tag=f"vb{p}",
                         name=f"vb{p}") for p in range(NP)]
        yt = [const.tile([128, T], BF16, tag=f"yt{p}", name=f"yt{p}")
              for p in range(NP)]
        ident = const.tile([128, 128], BF16, tag="ident")
        ones = const.tile([128, 128], BF16, tag="ones")

        wqv = wq.rearrange("p (m k t) -> p m k t", m=NM, k=KC)
        xbv = xb.rearrange("p (c k t) -> p c k t", c=NCH, k=KC)
        # the first qkv m-group needs wq m0 + xb ch0; split ch0 by kc halves
        # so the PE can start after ~1.5MB of transfer
        nc.sync.dma_start(wq_sb[:, 0], wqv[:, 0])
        nc.sync.dma_start(xb_sb[:, 0, 0:4], xbv[:, 0, 0:4])
        nc.sync.dma_start(xb_sb[:, 0, 4:8], xbv[:, 0, 4:8])
        for m in range(1, NM):
            nc.sync.dma_start(wq_sb[:, m], wqv[:, m])
        for c in range(1, NCH):
            nc.sync.dma_start(xb_sb[:, c], xbv[:, c])
        nc.sync.dma_start(wp_sb[:], wp.rearrange("p (h t) -> p h t", h=NP))

        make_identity(nc, ident[:])
        nc.gpsimd.memset(ones[:], 1.0)
        # ones columns (denominator trick): col 65*j + 64 of every vb tile
        for p in range(NP):
            onesv = bass.AP(
                vb[p].tensor, vb[p][:].offset + HD,
                [vb[p][:].ap[0], [HD + 1, 32]],
            )
            nc.vector.tensor_copy(onesv, ones[:, 0:32])
        scale = 1.0 / float(np.sqrt(HD))

        def qkv_group(p, which, ch):
            # one m-group: 8-deep contraction into one PSUM bank + evacuation
            m = which * NP + p
            t0 = ch * NQ
            ps = psm.tile([128, NQ], F32, tag="ps")
            for kc in range(KC):
                nc.tensor.matmul(
                    ps[:],
                    wq_sb[:, m, kc, :],
                    xb_sb[:, ch, kc, :],
                    start=(kc == 0),
                    stop=(kc == KC - 1),
                )
            if which == 0:
                nc.vector.tensor_copy(qt[p][:, t0 : t0 + NQ], ps[:])
            elif which == 1:
                nc.vector.tensor_copy(kt[p][:, t0 : t0 + NQ], ps[:])
            else:
                vs = vspool.tile([128, NQ], BF16, tag="vs")
                nc.vector.tensor_copy(vs[:], ps[:])
                pt2 = psm.tile([128, NQ], BF16, tag="ps")
                for q in range(NG):
                    nc.tensor.transpose(
                        pt2[:, q * 128 : (q + 1) * 128],
                        vs[:, q * 128 : (q + 1) * 128],
                        ident[:],
                    )
                # pt2: [tok 128][tile q: h0 64 | h1 64] -> vb slots
                dstv = bass.AP(
                    vb[p].tensor,
                    vb[p][:].offset + ch * NG * VST,
                    [vb[p][:].ap[0], [VST, NG], [HD + 1, 2], [1, HD]],
                )
                srcv = pt2[:].rearrange("p (t h d) -> p t h d", t=NG, h=2)
                nc.vector.tensor_copy(dstv, srcv)

        def qkv_feed(p):
            # thunks for pair p's projection, Q/K first (attention consumes
            # them first), usable from jq 0
            return [
                (0, lambda w=w, c=c: qkv_group(p, w, c))
                for c in range(NCH)
                for w in (0, 1, 2)
            ]

        def finalize_norm(p, jq, ytu, avs=None):
            # divide O^T rows by the denominator row (broadcast to 64 parts);
            # the custom-DVE reciprocal and partition_broadcast only operate
            # on physical partition 0, so first move the denominator row
            # (partition 64) down with a plain scalar-engine copy
            q0 = jq * NQ
            dn0 = dnpool.tile([1, 2 * NQ], F32, tag="dn0")
            if avs is None:
                nc.scalar.copy(dn0[:], ytu[HD : HD + 1, :])
            else:
                # tail path: read the denominators straight from PSUM so the
                # scalar copy overlaps the vector ytu evacuation
                for h in range(2):
                    nc.scalar.copy(dn0[:, h * NQ : (h + 1) * NQ],
                                   avs[h][HD : HD + 1, :])
            dnr0 = dnpool.tile([1, 2 * NQ], F32, tag="dnr0")
            nc.vector.reciprocal_approx_fast(dnr0[:], dn0[:])
            dnr = dnpool.tile([HD, 2 * NQ], F32, tag="dnr")
            nc.gpsimd.partition_broadcast(dnr[:], dnr0[:])
            for h in range(2):
                nc.vector.tensor_mul(
                    yt[p][h * HD : (h + 1) * HD, q0 : q0 + NQ],
                    ytu[0:HD, h * NQ : (h + 1) * NQ],
                    dnr[:, h * NQ : (h + 1) * NQ],
                )

        def out_tile(tt):
            # one 128-token slice of the output projection
            os_ = ospool.tile([128, D], BF16, tag="os")
            for nn in range(D // NQ):
                pp = psm.tile([128, NQ], F32, tag="ps")
                for p in range(NP):
                    nc.tensor.matmul(
                        pp[:],
                        yt[p][:, tt * 128 : (tt + 1) * 128],
                        wp_sb[:, p, nn * NQ : (nn + 1) * NQ],
                        start=(p == 0),
                        stop=(p == NP - 1),
                    )
                nc.vector.tensor_copy(os_[:, nn * NQ : (nn + 1) * NQ], pp[:])
            nc.sync.dma_start(out[tt * 128 : (tt + 1) * 128, :], os_[:])

        def attention(p, feed, last=False):
            # Both heads of the pair processed together per kk-tile: h0 in
            # SBUF partitions 0-63, h1 in 64-127, so the S^T matmul pairs
            # land on PE row-tiles (64,128)@(0,0) and @(64,0) and overlap.
            # `feed` is a list of (min_jq, thunk): independent PE work
            # interleaved between kk steps to cover the exp latency.
            fi = 0
            gstep = 0
            nfeed = len(feed)
            for jq in range(NJQ):
                q0 = jq * NQ
                av0 = pav.tile([128, NQ], F32, tag="av0")
                av1 = pav.tile([128, NQ], F32, tag="av1")
                avs = [av0, av1]
                nkk = NG * (jq + 1)
                diag0 = NG * jq

                def s_exp(kk):
                    # S^T for both heads, then exp and the diagonal mask
                    i = kk - diag0          # >= 0 on the diagonal run
                    c0 = max(i, 0) * 128    # first valid q col in this chunk
                    w = NQ - c0
                    st = pst.tile([128, 2 * NQ], F32, tag="st")
                    for h in range(2):
                        nc.tensor.matmul(
                            st[:, h * NQ + c0 : (h + 1) * NQ],
                            kt[p][h * HD : (h + 1) * HD,
                                  kk * 128 : (kk + 1) * 128],
                            qt[p][h * HD : (h + 1) * HD, q0 + c0 : q0 + NQ],
                            start=True,
                            stop=True,
                        )
                    ptk = ptpool.tile([128, 2 * NQ], BF16, tag="pt")
                    stv = bass.AP(st.tensor, st[:].offset + c0,
                                  [st[:].ap[0], [NQ, 2], [1, w]])
                    ptv = bass.AP(ptk.tensor, ptk[:].offset + c0,
                                  [ptk[:].ap[0], [NQ, 2], [1, w]])
                    nc.scalar.activation(ptv, stv, EXP, scale=scale)
                    if i >= 0:
                        # zero q < kpart inside the 128-wide diagonal block
                        tri = bass.AP(ptk.tensor, ptk[:].offset + c0,
                                      [ptk[:].ap[0], [NQ, 2], [1, 128]])
                        nc.gpsimd.affine_select(
                            out=tri,
                            in_=tri,
                            pattern=[[0, 2], [1, 128]],
                            channel_multiplier=-1,
                            base=0,
                            compare_op=mybir.AluOpType.is_ge,
                            fill=0.0,
                        )
                    return ptk, c0

                # software pipeline: S(kk+1) is emitted before AV(kk) so the
                # PE has independent work while exp(kk) is in flight
                pend = s_exp(0)
                for kk in range(nkk):
                    nxt = s_exp(kk + 1) if kk + 1 < nkk else None
                    ptk, c0 = pend
                    for h in range(2):
                        nc.tensor.matmul(
                            avs[h][0 : HD + 1, c0:NQ],
                            vb[p][:, kk, h * (HD + 1) : (h + 1) * (HD + 1)],
                            ptk[:, h * NQ + c0 : (h + 1) * NQ],
                            start=(kk == 0),
                            stop=(kk == nkk - 1),
                        )
                    pend = nxt
                    gstep += 1
                    # pace the feed evenly over the 40 kk steps, honoring
                    # each thunk's earliest-jq window
                    while (fi < nfeed and feed[fi][0] <= jq
                           and (fi < min(6, gstep)
                                or (fi - 6) * (NSTEP - 6)
                                   <= (gstep - 6) * (nfeed - 6))):
                        feed[fi][1]()
                        fi += 1
                # evacuate PSUM promptly (frees the av slots); rows 0..63 are
                # the unnormalized O^T, row 64 the denominator
                ytu = ytupool.tile([HD + 1, 2 * NQ], BF16, tag="ytu")
                for h in range(2):
                    nc.vector.tensor_copy(
                        ytu[:, h * NQ : (h + 1) * NQ], avs[h][0 : HD + 1, :]
                    )
                finalize_norm(p, jq, ytu,
                              avs=avs if (last and jq == NJQ - 1) else None)
                # flush feed thunks whose window just opened
                while (fi < nfeed and feed[fi][0] <= jq + 1
                       and (fi < min(6, gstep)
                            or (fi - 6) * (NSTEP - 6)
                               <= (gstep - 6) * (nfeed - 6))):
                    feed[fi][1]()
                    fi += 1
            while fi < nfeed:
                feed[fi][1]()
                fi += 1

        # ---- schedule ----
        # prologue: pair 0's projection
        for (_, thunk) in qkv_feed(0):
            thunk()
        # pair p's attention carries pair p+1's projection; the last pair
        # carries the output projection (tt 4j..4j+3 opens once pair 3's
        # query chunk j is normalized)
        for p in range(NP - 1):
            attention(p, qkv_feed(p + 1))
        attention(
            NP - 1,
            [(tt // 4 + 1, lambda tt=tt: out_tile(tt)) for tt in range(T // 128)],
            last=True,
        )


def make_in_maps(x, w_attn, w_proj):
    """Pack full fp32 inputs into per-core bf16 input maps."""
    bf = ml_dtypes.bfloat16
    x = np.asarray(x, dtype=np.float32)
    w_attn = np.asarray(w_attn, dtype=np.float32)
    w_proj = np.asarray(w_proj, dtype=np.float32)

    xbs = []
    for b in range(B):
        xT = x[b].T  # [D, T]
        xbs.append(
            np.ascontiguousarray(
                xT.reshape(KC, 128, NCH, NQ).transpose(1, 2, 0, 3)
            ).reshape(128, NCH * KC * NQ).astype(bf)
        )

    in_maps = []
    for c in range(NCORES):
        b, g = c // 2, c % 2
        c0 = g * CW
        wsl = np.concatenate(
            [w_attn[:, c0 : c0 + CW],
             w_attn[:, D + c0 : D + c0 + CW],
             w_attn[:, 2 * D + c0 : 2 * D + c0 + CW]],
            axis=1,
        )  # [D, 3*CW]
        wq = np.ascontiguousarray(
            wsl.reshape(KC, 128, NM, 128).transpose(1, 2, 0, 3)
        ).reshape(128, NM * KC * 128).astype(bf)
        wpc = np.ascontiguousarray(
            w_proj[c0 : c0 + CW, :].reshape(NP, 128, D).transpose(1, 0, 2)
        ).reshape(128, NP * D).astype(bf)
        in_maps.append({"xb": xbs[b], "wq": wq, "wp": wpc})
    return in_maps


_NC_CACHE = None


def kernel(x: np.ndarray, w_attn: np.ndarray, w_proj: np.ndarray) -> np.ndarray:
    global _NC_CACHE
    if _NC_CACHE is None:
        _NC_CACHE = build_kernel()
    nc = _NC_CACHE

    in_maps = make_in_maps(x, w_attn, w_proj)
    res = run_bass_kernel_spmd(nc, in_maps, core_ids=list(range(NCORES)))
    y = np.empty((B, T, D), dtype=np.float32)
    for b in range(B):
        y[b] = np.asarray(res.results[2 * b]["out"], np.float32) + np.asarray(
            res.results[2 * b + 1]["out"], np.float32
        )
    return y


if __name__ == "__main__":
    inputs = {
        "x": np.random.randn(B, T, D).astype(np.float32),
        "w_attn": (np.random.randn(D, 3 * D) / np.sqrt(D)).astype(np.float32),
        "w_proj": (np.random.randn(D, D) / np.sqrt(D)).astype(np.float32),
    }
    y = kernel(**inputs)
    print(y.shape, y.dtype)
